# revision 9
# baseline (speedup 1.0000x reference)
"""BinaryLSTM (binary tree-LSTM cell) Trainium2 kernel.

Full-input contract: kernel(**inputs) takes the complete unsharded tensors and
returns (h, c), each [8192, 1024] float32, matching the reference.

Strategy
--------
Data-parallel over the batch dim: core r handles rows r*1024:(r+1)*1024.
The 14 weight matrices are fused on the host into per-gate blocks over the
concatenated input X = [p | hl | hr] ([B, 3072]).  Gate pre-activations are
computed as z[h, b] so the contraction dim sits on SBUF partitions:

  z_g[h, b] = sum_k Vg[k, h] * XT[k, b]   (lhsT = Vg tile, rhs = XT tile)

PSUM tiles are [h_part=128, b_free=512] and the per-gate bias (varying along
h) is a per-partition [128,1] bias fused into the ACT sigmoid/tanh.

Matmuls run in float16 (full PE rate, ~216ns per 128x128x512 MM vs ~227ns
for float32r; quantization error ~8e-4 on h, measured in simulation).  PSUM
accumulation and everything downstream is fp32.

The kernel is PE-bound (steady-state MM pitch is 215.8ns, back-to-back).
Ramp mitigations (the engine queues only start executing user instructions
at ~6-7.4us after a fixed framework preamble of barriers + register loads):
  * Dummy warm-up matmuls on a gpsimd-zeroed SBUF tile start at ~6.5us with
    no DMA dependency, so the HAM clock-gate ramp (PE starts at ~4/8 clock
    and needs ~5us of busy time to reach 8/8) overlaps the initial DMA wait
    instead of slowing the first real gemms.
  * The early DMAs are all ~256KB and issued in strict first-use order
    (pf-weights m=0, xt[0..7] per tile, pf-weights m=1..7, ...).  Each
    dma_start costs ~0.65us of sync-queue issue time and concurrent
    transfers share the ~358GB/s HBM port fairly, so a big out-of-order
    transfer delays everything behind it (measured: a 2MB block issued
    3rd starved the pf gemms for 8.5us and re-triggered the clock gate).

The shared forget p-projection (pf = p @ Wf.T) is computed once per tile
(K=1024 accumulation), copied PSUM->SBUF, and added to the two forget-gate
child projections (K=2048 each) on DVE — total 14*B*D*H MACs, the algorithmic
minimum.  All 8 pf blocks run first: they only need the first third of XT.
The o-gate is computed last so the c = i*u + fl*cl + fr*cr chain (DVE) and
tanh(c) (ACT) overlap its matmuls; the final o-tile eviction is chunked into
quarters so ACT/DVE/DMA pipeline on the tail.
"""

import os
import sys

for _p in ("/opt/trn_rl_repo", "/root/.axon_site/_ro/trn_rl_repo"):
    if os.path.isdir(_p) and _p not in sys.path:
        sys.path.append(_p)

import numpy as np

import concourse.bass as bass
import concourse.tile as tile
import concourse.mybir as mybir
from concourse import bacc
from concourse import bass_utils

B, D, H = 8192, 1024, 1024
NCORES = 8
BL = B // NCORES            # 1024 batch rows per core
K3 = 3 * D                  # 3072 contraction (p | hl | hr)
KT = K3 // 128              # 24 k-tiles
MT = H // 128               # 8 h-tiles (PSUM partition dim)
NFREE = 512                 # moving free dim per matmul (one PSUM bank, fp32)
NT = BL // NFREE            # 2 b-tiles
KC = 8                      # weight-chunk k-tiles per SBUF weight tile
NWARM = 5                   # clock-warm-up dummy matmuls

F32 = mybir.dt.float32
F16 = mybir.dt.float16

_CACHE = {}

# Results of the most recent hardware run (for test harness introspection).
LAST_RESULTS = None

# weight blocks: (name, #k-tiles, xt k-tile offset)
WKINDS = {
    "i": (KT, 0),      # input gate, full K
    "u": (KT, 0),      # candidate, full K
    "o": (KT, 0),      # output gate, full K
    "pf": (D // 128, 0),              # shared forget p-projection (p rows)
    "fl": (2 * D // 128, D // 128),   # f_left child projections (hl|hr rows)
    "fr": (2 * D // 128, D // 128),   # f_right child projections
}
# bias column index per activated gate
BIAS_IDX = {"i": 0, "fl": 1, "fr": 2, "u": 3, "o": 4}


def _build_program():
    nc = bacc.Bacc("TRN2", target_bir_lowering=False, debug=False,
                   num_devices=NCORES)

    xt_d = nc.dram_tensor("xt", [K3, BL], F16, kind="ExternalInput").ap()
    w_d = {}
    for kind, (nk, _) in WKINDS.items():
        w_d[kind] = nc.dram_tensor(f"w_{kind}", [MT, 128, nk * 128], F16,
                                   kind="ExternalInput").ap()
    clt_d = nc.dram_tensor("clt", [H, BL], F32, kind="ExternalInput").ap()
    crt_d = nc.dram_tensor("crt", [H, BL], F32, kind="ExternalInput").ap()
    bt_d = nc.dram_tensor("bt", [128, 5 * MT], F32, kind="ExternalInput").ap()
    ht_d = nc.dram_tensor("ht", [H, BL], F32, kind="ExternalOutput").ap()
    ct_d = nc.dram_tensor("ct", [H, BL], F32, kind="ExternalOutput").ap()

    SIG = mybir.ActivationFunctionType.Sigmoid
    TANH = mybir.ActivationFunctionType.Tanh

    with tile.TileContext(nc) as tc:
        with tc.tile_pool(name="const", bufs=1) as const_pool, \
             tc.tile_pool(name="xtp", bufs=KT) as xt_pool, \
             tc.tile_pool(name="wp", bufs=8) as w_pool, \
             tc.tile_pool(name="gp", bufs=1) as g_pool, \
             tc.tile_pool(name="ep", bufs=2) as e_pool, \
             tc.tile_pool(name="pp", bufs=8, space="PSUM") as p_pool:

            # ---- PE clock warm-up: dummy matmuls with no DMA dependency ----
            # gpsimd memset: the gpsimd queue clears its framework preamble
            # first (~5.9us), so the warm matmuls start ~1.5us earlier than
            # with a DVE memset.
            warm = const_pool.tile([128, NFREE], F16, name="warm", tag="warm")
            nc.gpsimd.memset(warm[:], 0.0)
            ps_warm = p_pool.tile([128, NFREE], F32, name="ps_warm", tag="ps")
            for wi in range(NWARM):
                nc.tensor.matmul(ps_warm[:], warm[:, 0:128], warm[:],
                                 start=(wi == 0), stop=(wi == NWARM - 1))

            xt_r = xt_d.rearrange("(k p) b -> p k b", p=128)
            xts = [None] * KT

            def load_x(k):
                x_t = xt_pool.tile([128, BL], F16, name=f"x_{k}", tag="x")
                nc.sync.dma_start(x_t[:], xt_r[:, k, :])
                xts[k] = x_t

            def xt_ap(k, n):
                """[128, NFREE] view of xt k-tile k, b-chunk n."""
                return xts[k][:, n * NFREE:(n + 1) * NFREE]

            def load_w(kind, m, pool=None, tag="w", eng=None):
                """Load the weight block for (kind, m) as KC-k-tile chunks."""
                nk, _ = WKINDS[kind]
                chunks = []
                for c in range(0, nk, KC):
                    t = (pool or w_pool).tile([128, KC, 128], F16,
                                              name=f"w_{kind}_{m}_{c}",
                                              tag=tag)
                    (eng or nc.sync).dma_start(
                        t[:],
                        w_d[kind][m][:, c * 128:(c + KC) * 128]
                        .rearrange("p (k c) -> p k c", k=KC))
                    chunks.append(t)
                return chunks

            # Early DMAs in strict first-use order, all <=256KB.  pf weights
            # live in const_pool (per-m tags) so they never contend with the
            # main-phase weight ring.  The first two loads (the first real
            # gemm's dependencies) go out on the gpsimd queue, whose
            # framework preamble clears ~1.4us before the sync queue's.
            w_pfs = [None] * MT
            w_pfs[0] = load_w("pf", 0, pool=const_pool, tag="wpf0",
                              eng=nc.gpsimd)
            x0_t = xt_pool.tile([128, BL], F16, name="x_0", tag="x")
            nc.gpsimd.dma_start(x0_t[:], xt_r[:, 0, :])
            xts[0] = x0_t
            for k in range(1, 8):
                load_x(k)
            for m in range(1, MT):
                w_pfs[m] = load_w("pf", m, pool=const_pool, tag=f"wpf{m}")

            bt_t = const_pool.tile([128, 5 * MT], F32, name="bt_t", tag="bt")
            nc.sync.dma_start(bt_t[:], bt_d)

            # m=0 forget-gate weights next (first thing the main phase
            # needs), then the rest of XT.
            w_fl0 = load_w("fl", 0)
            w_fr0 = load_w("fr", 0)
            for k in range(8, 16):
                load_x(k)
            for k in range(16, KT):
                load_x(k)

            def pf_w(m, k):
                return w_pfs[m][0][:, k, :]

            def gemm(kind, m, w_chunks, n_outer=False):
                """Accumulate the gate block, return NT psum tiles.

                n_outer=True finishes all of n=0 before starting n=1 so
                n=0's eviction chain overlaps n=1's matmuls (tail shave
                for the final gate).
                """
                nk, koff = WKINDS[kind]
                ps = [p_pool.tile([128, NFREE], F32,
                                  name=f"ps_{kind}_{m}_{n}", tag="ps")
                      for n in range(NT)]
                if n_outer:
                    for n in range(NT):
                        for k in range(nk):
                            nc.tensor.matmul(
                                ps[n][:], w_chunks[k // KC][:, k % KC, :],
                                xt_ap(koff + k, n),
                                start=(k == 0), stop=(k == nk - 1))
                else:
                    for k in range(nk):
                        w_t = w_chunks[k // KC]
                        for n in range(NT):
                            nc.tensor.matmul(
                                ps[n][:], w_t[:, k % KC, :],
                                xt_ap(koff + k, n),
                                start=(k == 0), stop=(k == nk - 1))
                return ps

            # Phase 1: all pf gemms (only consume XT k-tiles 0..7).  Their
            # SBUF results stay resident until each m's forget gates run.
            pf_sbs = []
            for m in range(MT):
                ps_pf = [p_pool.tile([128, NFREE], F32,
                                     name=f"ps_pf_{m}_{n}", tag="ps")
                         for n in range(NT)]
                for k in range(KC):
                    for n in range(NT):
                        nc.tensor.matmul(
                            ps_pf[n][:], pf_w(m, k), xt_ap(k, n),
                            start=(k == 0), stop=(k == KC - 1))
                pf_sb = []
                for n in range(NT):
                    t = g_pool.tile([128, NFREE], F32,
                                    name=f"pf_{m}_{n}", tag="pf",
                                    bufs=MT * NT)
                    nc.scalar.copy(t[:], ps_pf[n][:])
                    pf_sb.append(t)
                pf_sbs.append(pf_sb)

            for m in range(MT):
                pf_sb = pf_sbs[m]
                w_fl = w_fl0 if m == 0 else load_w("fl", m)
                w_fr = w_fr0 if m == 0 else load_w("fr", m)
                w_i = load_w("i", m)
                w_u = load_w("u", m)
                w_o = load_w("o", m)

                gates = {}
                for kind, w_t in (("fl", w_fl), ("fr", w_fr)):
                    ps = gemm(kind, m, w_t)
                    bi = BIAS_IDX[kind]
                    for n in range(NT):
                        z = e_pool.tile([128, NFREE], F32,
                                        name=f"z_{kind}_{m}_{n}",
                                        tag="zf")
                        nc.vector.tensor_add(z[:], ps[n][:], pf_sb[n][:])
                        gt = g_pool.tile([128, NFREE], F32,
                                         name=f"g_{kind}_{m}_{n}",
                                         tag=f"g{kind}", bufs=2)
                        nc.scalar.activation(
                            gt[:], z[:], SIG,
                            bias=bt_t[:, bi * MT + m: bi * MT + m + 1])
                        gates[(kind, n)] = gt

                ps_i = gemm("i", m, w_i)
                for n in range(NT):
                    gt = g_pool.tile([128, NFREE], F32,
                                     name=f"g_i_{m}_{n}", tag="gi", bufs=2)
                    nc.scalar.activation(
                        gt[:], ps_i[n][:], SIG,
                        bias=bt_t[:, 0 * MT + m: 0 * MT + m + 1])
                    gates[("i", n)] = gt

                ps_u = gemm("u", m, w_u)
                for n in range(NT):
                    gt = g_pool.tile([128, NFREE], F32,
                                     name=f"g_u_{m}_{n}", tag="gu", bufs=2)
                    nc.scalar.activation(
                        gt[:], ps_u[n][:], TANH,
                        bias=bt_t[:, 3 * MT + m: 3 * MT + m + 1])
                    gates[("u", n)] = gt

                # c-chain: independent of o, overlaps o's matmuls
                th_tiles = {}
                for n in range(NT):
                    sp = slice(m * 128, (m + 1) * 128)
                    sf = slice(n * NFREE, (n + 1) * NFREE)
                    cl_t = e_pool.tile([128, NFREE], F32,
                                       name=f"cl_{m}_{n}", tag="cl")
                    nc.sync.dma_start(cl_t[:], clt_d[sp, sf])
                    cr_t = e_pool.tile([128, NFREE], F32,
                                       name=f"cr_{m}_{n}", tag="cr")
                    nc.sync.dma_start(cr_t[:], crt_d[sp, sf])

                    iu = e_pool.tile([128, NFREE], F32,
                                     name=f"iu_{m}_{n}", tag="iu")
                    nc.vector.tensor_mul(iu[:], gates[("i", n)][:],
                                         gates[("u", n)][:])
                    fc1 = e_pool.tile([128, NFREE], F32,
                                      name=f"fc1_{m}_{n}", tag="fc1")
                    nc.vector.tensor_mul(fc1[:], gates[("fl", n)][:], cl_t[:])
                    fc2 = e_pool.tile([128, NFREE], F32,
                                      name=f"fc2_{m}_{n}", tag="fc2")
                    nc.vector.tensor_mul(fc2[:], gates[("fr", n)][:], cr_t[:])
                    # c accumulates in-place in iu
                    nc.vector.tensor_add(iu[:], iu[:], fc1[:])
                    nc.vector.tensor_add(iu[:], iu[:], fc2[:])
                    nc.sync.dma_start(ct_d[sp, sf], iu[:])

                    th = e_pool.tile([128, NFREE], F32,
                                     name=f"th_{m}_{n}", tag="th")
                    nc.scalar.activation(th[:], iu[:], TANH)
                    th_tiles[n] = th

                ps_o = gemm("o", m, w_o, n_outer=True)
                for n in range(NT):
                    sp = slice(m * 128, (m + 1) * 128)
                    # chunk the very last eviction so ACT/DVE/DMA pipeline
                    # on the kernel tail instead of running 512-wide serially
                    last = (m == MT - 1 and n == NT - 1)
                    nch = 2 if last else 1
                    cw = NFREE // nch
                    go = e_pool.tile([128, NFREE], F32,
                                     name=f"g_o_{m}_{n}", tag="go")
                    h_t = e_pool.tile([128, NFREE], F32,
                                      name=f"h_{m}_{n}", tag="h")
                    for ci in range(nch):
                        cs = slice(ci * cw, (ci + 1) * cw)
                        sf = slice(n * NFREE + ci * cw,
                                   n * NFREE + (ci + 1) * cw)
                        nc.scalar.activation(
                            go[:, cs], ps_o[n][:, cs], SIG,
                            bias=bt_t[:, 4 * MT + m: 4 * MT + m + 1])
                        nc.vector.tensor_mul(h_t[:, cs], go[:, cs],
                                             th_tiles[n][:, cs])
                        nc.sync.dma_start(ht_d[sp, sf], h_t[:, cs])

    nc.compile()
    return nc


def _get_program():
    if "nc" not in _CACHE:
        _CACHE["nc"] = _build_program()
    return _CACHE["nc"]


def _tile_weight(V, nk):
    """[nk*128, H] -> [MT, 128, nk*128] with [m][kp, k*128+mc] = V[k*128+kp, m*128+mc]."""
    return np.ascontiguousarray(
        V.reshape(nk, 128, MT, 128)
         .transpose(2, 1, 0, 3)
         .reshape(MT, 128, nk * 128)
         .astype(np.float16))


def kernel(hl, cl, hr, cr, p,
           Wd, Wdl, Wdr, bd,
           Wf, Wfll, Wflr, Wfrl, Wfrr, bfl, bfr,
           Wo, Wol, Wor, bo,
           Wi, Wil, Wir, bi):
    global LAST_RESULTS
    f32 = np.float32
    hl, cl, hr, cr, p = (np.asarray(a, dtype=f32) for a in (hl, cl, hr, cr, p))
    ws = {k: np.asarray(v, dtype=f32) for k, v in dict(
        Wd=Wd, Wdl=Wdl, Wdr=Wdr, Wf=Wf, Wfll=Wfll, Wflr=Wflr, Wfrl=Wfrl,
        Wfrr=Wfrr, Wo=Wo, Wol=Wol, Wor=Wor, Wi=Wi, Wil=Wil, Wir=Wir).items()}

    # Wf{gate l/r}{child l/r}: f_left mixes hl via Wfll and hr via Wflr;
    # f_right mixes hl via Wfrl and hr via Wfrr.
    wt = {
        "i": _tile_weight(np.concatenate(
            [ws["Wd"].T, ws["Wdl"].T, ws["Wdr"].T], 0), KT),
        "u": _tile_weight(np.concatenate(
            [ws["Wi"].T, ws["Wil"].T, ws["Wir"].T], 0), KT),
        "o": _tile_weight(np.concatenate(
            [ws["Wo"].T, ws["Wol"].T, ws["Wor"].T], 0), KT),
        "pf": _tile_weight(np.ascontiguousarray(ws["Wf"].T), 8),
        "fl": _tile_weight(np.concatenate(
            [ws["Wfll"].T, ws["Wflr"].T], 0), 16),
        "fr": _tile_weight(np.concatenate(
            [ws["Wfrl"].T, ws["Wfrr"].T], 0), 16),
    }

    Bt = np.empty((128, 5 * MT), dtype=f32)
    for name, b_ in (("i", bd), ("fl", bfl), ("fr", bfr), ("u", bi), ("o", bo)):
        gi = BIAS_IDX[name]
        Bt[:, gi * MT:(gi + 1) * MT] = np.asarray(b_, dtype=f32).reshape(MT, 128).T

    X = np.concatenate([p, hl, hr], axis=1)    # [B, 3D]

    in_maps = []
    for r in range(NCORES):
        rows = slice(r * BL, (r + 1) * BL)
        im = {
            "xt": np.ascontiguousarray(X[rows].T.astype(np.float16)),
            "clt": np.ascontiguousarray(cl[rows].T),
            "crt": np.ascontiguousarray(cr[rows].T),
            "bt": Bt,
        }
        for kind, arr in wt.items():
            im[f"w_{kind}"] = arr
        in_maps.append(im)

    nc = _get_program()
    res = bass_utils.run_bass_kernel_spmd(nc, in_maps,
                                          core_ids=list(range(NCORES)))
    LAST_RESULTS = res

    h = np.empty((B, H), dtype=f32)
    c = np.empty((B, H), dtype=f32)
    for r in range(NCORES):
        rows = slice(r * BL, (r + 1) * BL)
        h[rows] = res.results[r]["ht"].T
        c[rows] = res.results[r]["ct"].T
    return (h, c)
